# revision 1
# baseline (speedup 1.0000x reference)
"""Transformer block (pre-LN attention + MLP) for B=8, T=1024, C=768, H=12.

Sharding: pure data-parallel — one batch element per NeuronCore, identical
SPMD program on cores 0-7, no collectives.

Per-core dataflow (activations kept on-chip end to end):
  x -> LN1 (stats in natural layout, scale/bias folded into W_qkv on host)
    -> PE-transpose z1 -> z1^T [C, T]
    -> QKV: q^T,k^T [d, t] (head-major) and v natural [t, d] (+ones column)
    -> attention: S^T = K Q^T blocks (causal block-skip), exp on ACT (no
       max-subtraction; scores are O(1)), multiplicative staircase causal
       masks on DVE, AV accumulation -> unnormalized ctx^T + row-sums via the
       ones column, normalize with DVE reciprocal + gpsimd partition
       broadcast
    -> attn_proj back to natural layout + residual (x re-streamed from HBM)
    -> LN2 -> PE-transpose -> FC (+bias+gelu fused on ACT) -> MLP proj in two
       f-halves (SBUF accumulate) + residual -> y

Matmuls run as float32r (full PE rate at free-dim >= 256, ~19-bit mantissa).
"""

import threading
from contextlib import ExitStack

import numpy as np

import concourse.bass as bass
from concourse import bacc
import concourse.mybir as mybir
import concourse.tile as tile
from concourse.bass_utils import run_bass_kernel_spmd
from concourse.masks import make_identity


# ---------------------------------------------------------------------------

B, T, C, H, HD, F, P = 8, 1024, 768, 12, 64, 3072, 128
NT = T // P        # 8  t-chunks
NCC = C // P       # 6  c-chunks
NFH = F // 2 // P  # 12 f-chunks per half
EPS = 1e-5
ATTN_SCALE = 1.0 / 8.0

F32 = mybir.dt.float32
MM_DT = mybir.dt.float32r


def build_module(dbg: bool = False) -> bass.Bass:
    nc = bacc.Bacc()

    x_h = nc.dram_tensor("x", [T, C], F32, kind="ExternalInput")
    w1_h = nc.dram_tensor("w1", [C, 3 * C], F32, kind="ExternalInput")
    b1_h = nc.dram_tensor("b1", [3 * C], F32, kind="ExternalInput")
    wp_h = nc.dram_tensor("wp", [C, C], F32, kind="ExternalInput")
    bp_h = nc.dram_tensor("bp", [C], F32, kind="ExternalInput")
    w2_h = nc.dram_tensor("w2", [C, F], F32, kind="ExternalInput")
    b2_h = nc.dram_tensor("b2", [F], F32, kind="ExternalInput")
    wm_h = nc.dram_tensor("wm", [F, C], F32, kind="ExternalInput")
    bm_h = nc.dram_tensor("bm", [C], F32, kind="ExternalInput")
    y_h = nc.dram_tensor("y", [T, C], F32, kind="ExternalOutput")
    if dbg:
        dbg_z1t = nc.dram_tensor("dbg_z1t", [C, T], F32, kind="ExternalOutput")
        dbg_qkt = nc.dram_tensor("dbg_qkt", [2 * C, T], F32, kind="ExternalOutput")
        dbg_ctxt = nc.dram_tensor("dbg_ctxt", [C, T], F32, kind="ExternalOutput")
        dbg_x1 = nc.dram_tensor("dbg_x1", [T, C], F32, kind="ExternalOutput")

    def bcast_ap(ap1d, n_part=P):
        return bass.AP(
            tensor=ap1d.tensor, offset=ap1d.offset, ap=[[0, n_part], *ap1d.ap]
        )

    xr = x_h.ap().rearrange("(tc p) c -> p tc c", p=P)
    yr = y_h.ap().rearrange("(tc p) c -> p tc c", p=P)
    w1r = w1_h.ap().rearrange("(kc p) d -> p kc d", p=P)

    with tile.TileContext(nc) as tc, ExitStack() as top:
        consts = top.enter_context(tc.tile_pool(name="consts", bufs=1))

        # constants needed immediately (identity for LN1 transposes)
        ident = consts.tile([P, P], F32)
        make_identity(nc, ident[:])
        eps_t = consts.tile([P, 1], F32)
        nc.vector.memset(eps_t[:], EPS)
        ones_col = consts.tile([P, 1], F32)
        nc.vector.memset(ones_col[:], 1.0)
        scratch = consts.tile([P, 1], F32)
        # prefetch the Sqrt act-table while the first x chunk is in flight
        nc.scalar.activation(
            out=scratch[:], in_=eps_t[:],
            func=mybir.ActivationFunctionType.Sqrt, scale=1.0,
        )
        # allocated now, ops emitted later (off the startup critical path)
        b1qk = consts.tile([P, 2 * C // P], F32)
        b2c = consts.tile([P, F // P], F32)

        # ---- long-lived activation tensors (LIFO pool nesting) ------------
        x1p_ctx = ExitStack()
        x1p = x1p_ctx.enter_context(tc.tile_pool(name="x1p", bufs=1))
        # per-chunk tiles so attn_proj -> LN2 -> MLP-residual pipeline at
        # t-chunk granularity instead of whole-tensor dependencies
        x1 = [
            x1p.tile([P, C], F32, tag=f"x1_{i}", name=f"x1c_{i}")
            for i in range(NT)
        ]

        ctp_ctx = ExitStack()
        ctp = ctp_ctx.enter_context(tc.tile_pool(name="ctp", bufs=1))
        ctxT = [
            ctp.tile([P, T], MM_DT, tag=f"ctxT{cc}", name=f"ctxT_{cc}")
            for cc in range(NCC)
        ]

        wpp_ctx = ExitStack()
        wpp = wpp_ctx.enter_context(tc.tile_pool(name="wpp", bufs=1))
        wp_sb = wpp.tile([P, NCC, C], MM_DT)
        bp_b = wpp.tile([P, C], F32)

        qkv_ctx = ExitStack()
        qkp = qkv_ctx.enter_context(tc.tile_pool(name="qkp", bufs=1))
        vp = qkv_ctx.enter_context(tc.tile_pool(name="vp", bufs=1))
        qkT = qkp.tile([P, 2 * C // P, T], MM_DT)
        vnat = vp.tile([P, NT, H, HD + 1], MM_DT)
        b1v_b = vp.tile([P, C], F32)

        zt1_ctx = ExitStack()
        zt1p = zt1_ctx.enter_context(tc.tile_pool(name="zt1p", bufs=1))
        zt1 = zt1p.tile([P, NCC, T], MM_DT)

        w1_ctx = ExitStack()
        w1p = w1_ctx.enter_context(tc.tile_pool(name="w1p", bufs=7))

        def load_w1_pass(qk):
            tiles = []
            for kc in range(NCC):
                w1s = w1p.tile([P, C], MM_DT, tag="w1s", name=f"w1s_{qk}_{kc}")
                nc.sync.dma_start(
                    w1s[:], w1r[:, kc, qk * C : (qk + 1) * C].bitcast(MM_DT)
                )
                tiles.append(w1s)
            return tiles

        # ---- LN + transpose helper (copies split DVE/ACT) ------------------
        last_rstd = [None]

        def layernorm_transpose(get_src, zt_tile, zpool, spool, pst):
            for tci in range(NT):
                xt = get_src(tci)
                stats = spool.tile([P, 2, 6], F32, tag="stats")
                for s in range(2):
                    nc.vector.bn_stats(
                        out=stats[:, s, :], in_=xt[:, s * 384 : (s + 1) * 384]
                    )
                mv = spool.tile([P, 2], F32, tag="mv")
                nc.vector.bn_aggr(out=mv[:], in_=stats[:])
                rstd = spool.tile([P, 1], F32, tag="rstd")
                nc.scalar.activation(
                    out=rstd[:],
                    in_=mv[:, 1:2],
                    func=mybir.ActivationFunctionType.Sqrt,
                    bias=eps_t[:],
                    scale=1.0,
                )
                nc.vector.reciprocal(out=rstd[:], in_=rstd[:])
                last_rstd[0] = rstd
                z = zpool.tile([P, C], F32, tag="z")
                # normalize in two halves so the first transposes start
                # before the full row is done
                for hh in range(2):
                    nc.vector.tensor_scalar(
                        out=z[:, hh * 384 : (hh + 1) * 384],
                        in0=xt[:, hh * 384 : (hh + 1) * 384],
                        scalar1=mv[:, 0:1],
                        scalar2=rstd[:],
                        op0=mybir.AluOpType.subtract,
                        op1=mybir.AluOpType.mult,
                    )
                for cc in range(NCC):
                    pt = pst.tile([P, P], F32, tag="pt")
                    nc.tensor.transpose(
                        pt[:], z[:, cc * P : (cc + 1) * P], ident[:]
                    )
                    dst = zt_tile[:, cc, tci * P : (tci + 1) * P]
                    if cc % 3 == 0:
                        nc.vector.tensor_copy(out=dst, in_=pt[:])
                    else:
                        nc.scalar.copy(out=dst, in_=pt[:])

        # ---- stage 1a: LN1 (x streamed from HBM) --------------------------
        with (
            tc.tile_pool(name="xs1", bufs=2) as xs1,
            tc.tile_pool(name="ln_z", bufs=3) as _zp,
            tc.tile_pool(name="ln_s", bufs=6) as _sp,
            tc.tile_pool(name="ln_pt", bufs=6, space="PSUM") as _pp,
        ):
            def _ln1_src(tci):
                xt = xs1.tile([P, C], F32, tag="xt", name=f"xt1_{tci}")
                nc.sync.dma_start(xt[:], xr[:, tci, :])
                return xt[:]

            layernorm_transpose(_ln1_src, zt1, _zp, _sp, _pp)
            w1q = load_w1_pass(0)

        # deferred const loads (keep the sync-DMA queue clear at startup)
        nc.sync.dma_start(
            b1qk[:], b1_h.ap()[0 : 2 * C].rearrange("(dc p) -> p dc", p=P)
        )
        nc.sync.dma_start(b2c[:], b2_h.ap().rearrange("(fc p) -> p fc", p=P))
        nc.gpsimd.dma_start(b1v_b[:], bcast_ap(b1_h.ap()[2 * C : 3 * C]))
        nc.gpsimd.dma_start(bp_b[:], bcast_ap(bp_h.ap()))
        for kc in range(NCC):
            nc.sync.dma_start(
                wp_sb[:, kc, :],
                wp_h.ap()
                .rearrange("(kc p) c -> p kc c", p=P)[:, kc, :]
                .bitcast(MM_DT),
            )

        # ---- stage 1b: QKV -------------------------------------------------
        with (
            tc.tile_pool(name="ps_qk", bufs=4, space="PSUM") as ps_qk,
            tc.tile_pool(name="ps_v", bufs=2, space="PSUM") as ps_v,
        ):
            for qk in range(2):
                w1s = w1q if qk == 0 else load_w1_pass(1)
                for dcl in range(NCC):
                    dc = qk * NCC + dcl
                    for j in range(2):
                        ps = ps_qk.tile([P, 512], F32, tag="psqk")
                        for kc in range(NCC):
                            nc.tensor.matmul(
                                ps[:],
                                (w1s[kc][:, dcl * P : (dcl + 1) * P]),
                                (zt1[:, kc, j * 512 : (j + 1) * 512]),
                                start=(kc == 0),
                                stop=(kc == NCC - 1),
                            )
                        nc.vector.tensor_scalar_add(
                            out=qkT[:, dc, j * 512 : (j + 1) * 512],
                            in0=ps[:],
                            scalar1=b1qk[:, dc : dc + 1],
                        )
            # v columns -> natural layout (+bias, +ones col)
            w1v = load_w1_pass(2)
            for tci in range(NT):
                psv = ps_v.tile([P, C], F32, tag="psv")
                for kc in range(NCC):
                    nc.tensor.matmul(
                        psv[:, 0:512],
                        (zt1[:, kc, tci * P : (tci + 1) * P]),
                        (w1v[kc][:, 0:512]),
                        start=(kc == 0),
                        stop=(kc == NCC - 1),
                    )
                    nc.tensor.matmul(
                        psv[:, 512:768],
                        (zt1[:, kc, tci * P : (tci + 1) * P]),
                        (w1v[kc][:, 512:768]),
                        start=(kc == 0),
                        stop=(kc == NCC - 1),
                    )
                nc.vector.tensor_add(
                    out=vnat[:, tci, 0:8, 0:HD],
                    in0=psv[:, 0:512].rearrange("p (h d) -> p h d", h=8),
                    in1=b1v_b[:, 0:512].rearrange("p (h d) -> p h d", h=8),
                )
                nc.vector.tensor_add(
                    out=vnat[:, tci, 8:12, 0:HD],
                    in0=psv[:, 512:768].rearrange("p (h d) -> p h d", h=4),
                    in1=b1v_b[:, 512:768].rearrange("p (h d) -> p h d", h=4),
                )
                nc.vector.tensor_copy(
                    out=vnat[:, tci, :, HD : HD + 1].rearrange(
                        "p h one -> p (h one)"
                    ),
                    in_=ones_col[:].to_broadcast((P, H)),
                )

        if dbg:
            nc.sync.dma_start(
                dbg_z1t.ap().rearrange("(cc p) t -> p cc t", p=P).bitcast(MM_DT),
                zt1[:],
            )
            nc.sync.dma_start(
                dbg_qkt.ap().rearrange("(dc p) t -> p dc t", p=P).bitcast(MM_DT),
                qkT[:],
            )

        w1_ctx.close()
        zt1_ctx.close()

        cm_ctx = ExitStack()
        cmp_ = cm_ctx.enter_context(tc.tile_pool(name="cmp", bufs=1))
        cmask = cmp_.tile([P, P], F32)
        # prefetch the Exp act-table during the QKV tail
        nc.scalar.activation(
            out=scratch[:], in_=last_rstd[0][:],
            func=mybir.ActivationFunctionType.Exp, scale=1.0,
        )
        # one shared lower-triangle mask: every diagonal block's live
        # triangle at columns [r, r+128) is the same relative pattern
        # (keep 1.0 where q - r >= k)
        nc.vector.memset(cmask[:], 1.0)
        nc.gpsimd.affine_select(
            out=cmask[:],
            in_=cmask[:],
            compare_op=mybir.AluOpType.is_ge,
            fill=0.0,
            base=0,
            pattern=[[1, P]],
            channel_multiplier=-1,
        )

        last_eS = [None]
        # ---- stage 2: attention -------------------------------------------
        with (
            tc.tile_pool(name="esp", bufs=3) as esp,
            tc.tile_pool(name="rcp", bufs=3) as rcp,
            tc.tile_pool(name="bcp", bufs=3) as bcp,
            tc.tile_pool(name="ps_s", bufs=3, space="PSUM") as ps_s,
            tc.tile_pool(name="ps_c", bufs=2, space="PSUM") as ps_c,
        ):
            for j_h in range(2 * H):
                j, h = divmod(j_h, H)
                row = (h % 2) * HD
                qT_h = qkT[row : row + HD, h // 2, :]
                kT_h = qkT[row : row + HD, NCC + h // 2, :]
                if True:
                    nm = 4 * (j + 1)
                    eS = esp.tile([P, 8, 512], MM_DT, tag="eS")
                    last_eS[0] = eS
                    # full (non-diagonal) k-chunks, exp batched in pairs
                    for half in range(2 * j):
                        psS = ps_s.tile([P, 2, 512], F32, tag="psS")
                        for mi in range(2):
                            m = half * 2 + mi
                            nc.tensor.matmul(
                                psS[:, mi, :],
                                (kT_h[:, m * P : (m + 1) * P]),
                                (qT_h[:, j * 512 : (j + 1) * 512]),
                                start=True,
                                stop=True,
                            )
                        nc.scalar.activation(
                            out=eS[:, half * 2 : half * 2 + 2, :],
                            in_=psS[:],
                            func=mybir.ActivationFunctionType.Exp,
                            scale=ATTN_SCALE,
                        )
                    # diagonal-crossing k-chunks: only live columns q >= r
                    for di in range(2):
                        psS = ps_s.tile([P, 2, 512], F32, tag="psS")
                        for mi in range(2):
                            m = 4 * j + di * 2 + mi
                            r = m * P - 512 * j
                            nc.tensor.matmul(
                                psS[:, mi, r:512],
                                (kT_h[:, m * P : (m + 1) * P]),
                                (qT_h[:, j * 512 + r : (j + 1) * 512]),
                                start=True,
                                stop=True,
                            )
                        for mi in range(2):
                            m = 4 * j + di * 2 + mi
                            r = m * P - 512 * j
                            # columns [0, r) are never read: the AV matmul
                            # streams only [r:512] of this block
                            nc.scalar.activation(
                                out=eS[:, m, r:512],
                                in_=psS[:, mi, r:512],
                                func=mybir.ActivationFunctionType.Exp,
                                scale=ATTN_SCALE,
                            )
                            nc.vector.tensor_mul(
                                out=eS[:, m, r : r + P],
                                in0=eS[:, m, r : r + P].bitcast(F32),
                                in1=cmask[:],
                            )
                    psC = ps_c.tile([HD + 1, 512], F32, tag="psC")
                    for m in range(nm):
                        # columns q < r of diagonal blocks are zero in eS;
                        # skip streaming them (m == 0 writes full width, so
                        # has_written covers the bank before partial adds)
                        r = max(0, m * P - 512 * j)
                        nc.tensor.matmul(
                            psC[:, r:512],
                            (vnat[:, m, h, :]),
                            (eS[:, m, r:512]),
                            start=(m == 0),
                            stop=(m == nm - 1),
                        )
                    recip = rcp.tile([1, 512], F32, tag="recip")
                    nc.vector.reciprocal(out=recip[:], in_=psC[HD : HD + 1, :])
                    bc = bcp.tile([HD, 512], F32, tag="bc")
                    nc.gpsimd.partition_broadcast(bc[:], recip[:])
                    nc.vector.tensor_mul(
                        out=ctxT[h // 2][row : row + HD, j * 512 : (j + 1) * 512],
                        in0=psC[0:HD, :],
                        in1=bc[:],
                    )

            # prefetch the Sqrt table for LN2 during the attention tail (the
            # input dep on the last eS keeps it after the attention exps)
            nc.scalar.activation(
                out=scratch[:], in_=last_eS[0][:, 7, 0:1].bitcast(F32),
                func=mybir.ActivationFunctionType.Sqrt, scale=1.0,
            )

        cm_ctx.close()
        qkv_ctx.close()

        if dbg:
            for cc in range(NCC):
                nc.sync.dma_start(
                    dbg_ctxt.ap()
                    .rearrange("(cc p) t -> p cc t", p=P)[:, cc, :]
                    .bitcast(MM_DT),
                    ctxT[cc][:],
                )

        # ---- stage 3: attention projection + residual ---------------------
        with (
            tc.tile_pool(name="xs2", bufs=4) as xs2,
            tc.tile_pool(name="ps_ap", bufs=3, space="PSUM") as ps_ap,
        ):
            for tci in range(NT):
                xt2 = xs2.tile([P, C], F32, tag="xt2", name=f"xt2_{tci}")
                nc.sync.dma_start(xt2[:], xr[:, tci, :])
                ps = ps_ap.tile([P, C], F32, tag="psap")
                for kc in range(NCC):
                    nc.tensor.matmul(
                        ps[:, 0:512],
                        (ctxT[kc][:, tci * P : (tci + 1) * P]),
                        (wp_sb[:, kc, 0:512]),
                        start=(kc == 0),
                        stop=(kc == NCC - 1),
                    )
                    nc.tensor.matmul(
                        ps[:, 512:768],
                        (ctxT[kc][:, tci * P : (tci + 1) * P]),
                        (wp_sb[:, kc, 512:768]),
                        start=(kc == 0),
                        stop=(kc == NCC - 1),
                    )
                nc.vector.tensor_add(out=x1[tci][:], in0=ps[:], in1=xt2[:])
                nc.vector.tensor_add(
                    out=x1[tci][:], in0=x1[tci][:], in1=bp_b[:]
                )

        wpp_ctx.close()
        ctp_ctx.close()

        if dbg:
            for i in range(NT):
                nc.sync.dma_start(
                    dbg_x1.ap().rearrange("(tc p) c -> p tc c", p=P)[:, i, :],
                    x1[i][:],
                )

        # ---- stage 4+5: LN2 + MLP -----------------------------------------
        w2_ctx = ExitStack()
        w2p = w2_ctx.enter_context(tc.tile_pool(name="w2p", bufs=6))
        w2r = w2_h.ap().rearrange("(kc p) f -> p kc f", p=P)
        wmr = wm_h.ap().rearrange("(fc p) c -> p fc c", p=P)

        def load_w2_half(half):
            tiles = []
            for kc in range(NCC):
                w2s = w2p.tile(
                    [P, F // 2], MM_DT, tag="w2s", name=f"w2s_{half}_{kc}"
                )
                nc.sync.dma_start(
                    w2s[:],
                    w2r[:, kc, half * (F // 2) : (half + 1) * (F // 2)].bitcast(
                        MM_DT
                    ),
                )
                tiles.append(w2s)
            return tiles

        w2h0 = load_w2_half(0)

        zt2_ctx = ExitStack()
        zt2p = zt2_ctx.enter_context(tc.tile_pool(name="zt2p", bufs=1))
        zt2 = zt2p.tile([P, NCC, T], MM_DT)
        with (
            tc.tile_pool(name="ln2_z", bufs=3) as _zp,
            tc.tile_pool(name="ln2_s", bufs=6) as _sp,
            tc.tile_pool(name="ln2_pt", bufs=6, space="PSUM") as _pp,
        ):
            layernorm_transpose(lambda tci: x1[tci][:], zt2, _zp, _sp, _pp)

        # prefetch the Gelu table while the FC matmuls accumulate (input
        # dep on LN2's last rstd keeps it after the LN2 sqrts)
        nc.scalar.activation(
            out=scratch[:], in_=last_rstd[0][:],
            func=mybir.ActivationFunctionType.Gelu_apprx_tanh, scale=1.0,
        )
        with (
            tc.tile_pool(name="mlpc", bufs=1) as mlpc,
            tc.tile_pool(name="gtp", bufs=1) as gtp,
            tc.tile_pool(name="wmp", bufs=12) as wmp,
            tc.tile_pool(name="ps_fc", bufs=2, space="PSUM") as ps_fc,
            tc.tile_pool(name="ps_mlp", bufs=3, space="PSUM") as ps_mlp,
        ):
            bm_b = mlpc.tile([P, C], F32)
            nc.gpsimd.dma_start(bm_b[:], bcast_ap(bm_h.ap()))

            def load_wm_half(half):
                tiles = []
                for kc in range(NFH):
                    wms = wmp.tile(
                        [P, C], MM_DT, tag="wms", name=f"wms_{half}_{kc}"
                    )
                    nc.sync.dma_start(
                        wms[:], wmr[:, half * NFH + kc, :].bitcast(MM_DT)
                    )
                    tiles.append(wms)
                return tiles

            for half in range(2):
                w2s = w2h0 if half == 0 else load_w2_half(1)
                wms = load_wm_half(half)
                gT = gtp.tile([P, NFH, T], MM_DT, tag="gT", name=f"gT_{half}")
                for mf in range(NFH):
                    fc_glob = half * NFH + mf
                    for j in range(2):
                        ps = ps_fc.tile([P, 512], F32, tag="psfc")
                        for kc in range(NCC):
                            nc.tensor.matmul(
                                ps[:],
                                (w2s[kc][:, mf * P : (mf + 1) * P]),
                                (zt2[:, kc, j * 512 : (j + 1) * 512]),
                                start=(kc == 0),
                                stop=(kc == NCC - 1),
                            )
                        nc.scalar.activation(
                            out=gT[:, mf, j * 512 : (j + 1) * 512],
                            in_=ps[:],
                            func=mybir.ActivationFunctionType.Gelu_apprx_tanh,
                            bias=b2c[:, fc_glob : fc_glob + 1],
                            scale=1.0,
                        )
                for grp in ((0, 1, 2), (3, 4, 5), (6, 7)):
                    pss = {}
                    for tci in grp:
                        psm = ps_mlp.tile(
                            [P, C], F32, tag="psmlp", name=f"psm_{half}_{tci}"
                        )
                        pss[tci] = psm
                    for kc in range(NFH):
                        for tci in grp:
                            nc.tensor.matmul(
                                pss[tci][:, 0:512],
                                (gT[:, kc, tci * P : (tci + 1) * P]),
                                (wms[kc][:, 0:512]),
                                start=(kc == 0),
                                stop=(kc == NFH - 1),
                            )
                            nc.tensor.matmul(
                                pss[tci][:, 512:768],
                                (gT[:, kc, tci * P : (tci + 1) * P]),
                                (wms[kc][:, 512:768]),
                                start=(kc == 0),
                                stop=(kc == NFH - 1),
                            )
                    for tci in grp:
                        nc.vector.tensor_add(
                            out=x1[tci][:], in0=x1[tci][:], in1=pss[tci][:]
                        )
                        if half == 0:
                            nc.vector.tensor_add(
                                out=x1[tci][:], in0=x1[tci][:], in1=bm_b[:]
                            )
                        else:
                            nc.sync.dma_start(yr[:, tci, :], x1[tci][:])

        zt2_ctx.close()
        w2_ctx.close()
        x1p_ctx.close()

    nc.compile()
    return nc


# ---------------------------------------------------------------------------
# host wrapper
# ---------------------------------------------------------------------------

_module_cache: dict = {}
_module_lock = threading.Lock()


def _get_module(dbg: bool = False) -> bass.Bass:
    with _module_lock:
        if dbg not in _module_cache:
            _module_cache[dbg] = build_module(dbg)
        return _module_cache[dbg]


def _fold_inputs(
    x, ln1_scale, ln1_bias, w_qkv, b_qkv, w_attn_proj, b_attn_proj,
    ln2_scale, ln2_bias, w_fc, b_fc, w_mlp_proj, b_mlp_proj,
):
    f32 = np.float32
    w1 = (ln1_scale[:, None].astype(np.float64) * w_qkv.astype(np.float64)).astype(f32)
    b1 = (b_qkv.astype(np.float64) + ln1_bias.astype(np.float64) @ w_qkv.astype(np.float64)).astype(f32)
    w2 = (ln2_scale[:, None].astype(np.float64) * w_fc.astype(np.float64)).astype(f32)
    b2 = (b_fc.astype(np.float64) + ln2_bias.astype(np.float64) @ w_fc.astype(np.float64)).astype(f32)
    shared = {
        "w1": np.ascontiguousarray(w1),
        "b1": np.ascontiguousarray(b1),
        "wp": np.ascontiguousarray(w_attn_proj.astype(f32)),
        "bp": np.ascontiguousarray(b_attn_proj.astype(f32)),
        "w2": np.ascontiguousarray(w2),
        "b2": np.ascontiguousarray(b2),
        "wm": np.ascontiguousarray(w_mlp_proj.astype(f32)),
        "bm": np.ascontiguousarray(b_mlp_proj.astype(f32)),
    }
    return [
        {"x": np.ascontiguousarray(x[b].astype(f32)), **shared} for b in range(B)
    ]


def run(inputs: dict, dbg: bool = False, **spmd_kwargs):
    """Run on 8 cores; returns BassKernelResults."""
    args = {k: np.asarray(v) for k, v in inputs.items()}
    in_maps = _fold_inputs(
        args["x"], args["ln1_scale"], args["ln1_bias"], args["w_qkv"],
        args["b_qkv"], args["w_attn_proj"], args["b_attn_proj"],
        args["ln2_scale"], args["ln2_bias"], args["w_fc"], args["b_fc"],
        args["w_mlp_proj"], args["b_mlp_proj"],
    )
    nc = _get_module(dbg)
    res = run_bass_kernel_spmd(nc, in_maps, core_ids=list(range(B)), **spmd_kwargs)
    return res


def kernel(**inputs) -> np.ndarray:
    res = run(inputs)
    return np.stack([res.results[b]["y"] for b in range(B)], axis=0).astype(
        np.float32
    )


if __name__ == "__main__":
    build_module(dbg=False)
    print("module built OK")

